# revision 16
# baseline (speedup 1.0000x reference)
"""Bass/Trainium2 kernel for nn_BootstrappedCE.

Math note: the reference computes per-pixel CE, masks it per class,
and takes per-class top-k with k = 0.15*B*H*W = 314572.  Each class
has ~B*H*W/C = 262144 nonzero (strictly positive) CE entries, which is
< k, so top-k keeps *all* nonzero entries plus zero padding.  Hence

    mean(topk) == sum(ce over all pixels) / (C * k)

whenever every per-class pixel count <= k (verified on the host from
the actual target tensor; exact host fallback otherwise).

Device work per core (data-parallel over batch, 1 batch/core):
    ce[p] = sum_{f} [ log(sum_c exp(x_c)) - x_{target} ]
computed as 8x Exp (ACT) -> pairwise-tree sum (DVE, bf16) -> Ln with
free-axis accum (ACT) -> 8x scalar_tensor_tensor((t==c)*x_c, accum)
(DVE) -> per-partition ce[128,1] DMA'd out; host sums partitions/cores.

Hardware constraint that shapes the structure: every TPB instruction
carries at most ONE semaphore wait, and the Tile tail-drain takes one
wait per active proc (engine or DMA lane).  So we keep procs at 8:
ACT + DVE + 6 SWDGE DMA lanes (T, 4x X-pair loads, 1 store).
"""

import numpy as np

_B, _C, _H, _W = 8, 8, 512, 512
_N = _H * _W            # 262144 pixels per batch
_P, _F = 128, 2048      # _N == _P * _F
_NCORES = 8

_nc_cache = None
_last_results = None    # test.py introspection
_TRACE = False
_TRACE_KWARGS = {}


def _build_nc():
    import concourse.tile as tile
    from concourse import bacc, mybir

    f32 = mybir.dt.float32
    bf16 = mybir.dt.bfloat16
    AF = mybir.ActivationFunctionType
    OP = mybir.AluOpType
    AX = mybir.AxisListType

    nc = bacc.Bacc("TRN2", target_bir_lowering=False)
    x = nc.dram_tensor("x", [_C, _P, _F], f32, kind="ExternalInput")
    t = nc.dram_tensor("t", [_P, _F], f32, kind="ExternalInput")
    res = nc.dram_tensor("res", [_P, 1], f32, kind="ExternalOutput")

    with tile.TileContext(nc) as tc:
        with (
            tc.tile_pool(name="xin", bufs=1) as xin,
            tc.tile_pool(name="work", bufs=1) as work,
            tc.tile_pool(name="junk", bufs=8) as junkp,
        ):
            X = xin.tile([_P, _C, _F], f32)
            T = xin.tile([_P, _F], f32)
            E = work.tile([_P, _C, _F], bf16)
            TR = work.tile([_P, 4, _F], bf16)
            S = work.tile([_P, _F], bf16)
            LSE = work.tile([_P, _F], bf16)
            lse_acc = work.tile([_P, 1], f32)
            pick_acc = work.tile([_P, _C], f32)
            pick_tot = work.tile([_P, 1], f32)
            ce = work.tile([_P, 1], f32)

            nc.sync.dma_start(out=T, in_=t[:, :])
            for c in range(_C):
                nc.sync.dma_start(out=X[:, c, :], in_=x[c, :, :])

            # E_c = exp(X_c) on the scalar (ACT) engine, bf16 out.
            for c in range(_C):
                nc.scalar.activation(out=E[:, c, :], in_=X[:, c, :], func=AF.Exp)

            # S = sum_c E_c: pairwise tree, DVE bf16 (2x mode).
            for i in range(4):
                nc.vector.tensor_add(
                    out=TR[:, i, :], in0=E[:, 2 * i, :], in1=E[:, 2 * i + 1, :]
                )
            nc.vector.tensor_add(out=TR[:, 0, :], in0=TR[:, 0, :], in1=TR[:, 1, :])
            nc.vector.tensor_add(out=TR[:, 2, :], in0=TR[:, 2, :], in1=TR[:, 3, :])
            nc.vector.tensor_add(out=S, in0=TR[:, 0, :], in1=TR[:, 2, :])

            # lse_acc[p] = sum_f log(S[p,f])
            nc.scalar.activation(
                out=LSE, in_=S, func=AF.Ln, accum_out=lse_acc[:, 0:1]
            )

            # Joiner: pull the T-DMA wait onto a copy (the STT struct has
            # room for only one sync wait; DVE per-proc clock then covers T
            # for all following STTs, which each only wait on their X DMA).
            tjoin = work.tile([_P, 1], f32)
            nc.vector.tensor_copy(out=tjoin, in_=T[:, 0:1])

            # pick_acc[p,c] = sum_f (T==c) * X_c
            for c in range(_C):
                junk = junkp.tile([_P, _F], bf16, tag="junk")
                nc.vector.scalar_tensor_tensor(
                    out=junk,
                    in0=T,
                    scalar=float(c),
                    in1=X[:, c, :],
                    op0=OP.is_equal,
                    op1=OP.mult,
                    accum_out=pick_acc[:, c : c + 1],
                )

            nc.vector.tensor_reduce(out=pick_tot, in_=pick_acc, axis=AX.X, op=OP.add)
            nc.vector.tensor_sub(out=ce, in0=lse_acc, in1=pick_tot)
            nc.sync.dma_start(out=res[:, :], in_=ce[:, 0:1])
    nc.compile()
    return nc


def _exact_host(output, target):
    """Bit-faithful numpy fallback of the reference (only used if the
    top-k == all-nonzeros identity does not hold for these inputs)."""
    x = output.astype(np.float64)
    B, C, H, W = x.shape
    k = int(0.15 * B * H * W)
    m = x.max(axis=1, keepdims=True)
    lse = np.log(np.exp(x - m).sum(axis=1, keepdims=True)) + m
    tt = target.astype(np.int64)
    xt = np.take_along_axis(x, tt[:, None, :, :], axis=1)
    ce = (lse - xt)[:, 0].reshape(B, -1)
    tflat = tt.reshape(B, -1)
    total = 0.0
    n = B * H * W
    for c in range(C):
        row = np.where(tflat == c, ce, 0.0).reshape(-1)
        total += np.partition(row, n - k)[n - k :].sum()
    return np.float32(total / (C * k))


def kernel(output, target):
    output = np.ascontiguousarray(np.asarray(output), dtype=np.float32)
    target_i = np.asarray(target).astype(np.int64)
    B, C, H, W = output.shape
    k = int(0.15 * B * H * W)
    counts = np.bincount(target_i.ravel(), minlength=C)
    if (B, C, H, W) != (_B, _C, _H, _W) or counts.max() > k:
        return _exact_host(output, target_i)

    global _nc_cache, _last_results
    if _nc_cache is None:
        _nc_cache = _build_nc()

    from concourse.bass_utils import run_bass_kernel_spmd

    xs = output.reshape(B, C, _P, _F)
    ts = target_i.reshape(B, _P, _F).astype(np.float32)
    in_maps = [
        {"x": np.ascontiguousarray(xs[b]), "t": np.ascontiguousarray(ts[b])}
        for b in range(B)
    ]
    r = run_bass_kernel_spmd(
        _nc_cache,
        in_maps,
        core_ids=list(range(_NCORES)),
        trace=_TRACE,
        **_TRACE_KWARGS,
    )
    _last_results = r
    total = float(
        np.sum([rr["res"][:, 0].astype(np.float64).sum() for rr in r.results])
    )
    return np.float32(total / (C * k))


# revision 48
# speedup vs baseline: 66321.0448x; 66321.0448x over previous
"""Bass/Trainium2 kernel for nn_BootstrappedCE.

Math note: the reference computes per-pixel CE, masks it per class,
and takes per-class top-k with k = 0.15*B*H*W = 314572.  Each class
has ~B*H*W/C = 262144 nonzero (strictly positive) CE entries, which is
< k, so top-k keeps *all* nonzero entries plus zero padding.  Hence

    mean(topk) == sum(ce over all pixels) / (C * k)

whenever every per-class pixel count <= k (verified on the host from
the actual target tensor; exact host fallback otherwise).

Device work per core (data-parallel over batch, 1 batch/core):
    ce[p] = sum_f [ log(sum_c exp(x_c)) - x_{target} ]
Split into column chunks for DMA/compute overlap.  Per chunk:
8x Exp (ACT, bf16 out) -> linear-chain sum (DVE bf16 2x) -> Ln with
free-axis accum (ACT) -> 8x scalar_tensor_tensor((t==c)*x_c, accum)
split DVE/GPSIMD.  Host sums the per-partition partials (unshard).
"""

import numpy as np

_B, _C, _H, _W = 8, 8, 512, 512
_N = _H * _W            # 262144 pixels per batch
_P, _F = 128, 2048      # _N == _P * _F
_CHUNKS = [896, 640, 512]       # column widths (sum = _F)
_EXP_GROUPS = [1, 1, 1]         # classes per Exp op
_NCHUNK = len(_CHUNKS)
_NCORES = 8

_nc_cache = None
_last_results = None    # test.py introspection
_TRACE = False
_TRACE_KWARGS = {}


def _build_nc():
    import bass_rust as _bass_rust
    import concourse.tile as tile
    from concourse import bacc, mybir
    from concourse.hw_specs import get_activation_tables
    from concourse.tile_rust import add_dep_helper

    f32 = mybir.dt.float32
    bf16 = mybir.dt.bfloat16
    AF = mybir.ActivationFunctionType
    OP = mybir.AluOpType
    AX = mybir.AxisListType

    class _Bacc(bacc.Bacc):
        # Force Exp AND Ln onto the one set that holds both, so the
        # chunked Exp<->Ln alternation doesn't reload tables 5x.  The
        # emitted act_func_set_id is an INDEX into this list, so blank
        # the competing sets' function lists instead of reordering.
        def insert_act_table_loads(self):
            tables = list(get_activation_tables(self.m.arch).items())
            blank = {"exp_and_others", "exp_and_friends", "natural_log"}
            tables = [(n, set() if n in blank else f) for n, f in tables]
            _bass_rust.insert_act_table_loads(self, tables)

    nc = _Bacc("TRN2", target_bir_lowering=False)
    x = nc.dram_tensor("x", [_C, _P, _F], f32, kind="ExternalInput")
    f8 = mybir.dt.float8e4
    t = nc.dram_tensor("t", [_P, _F], f8, kind="ExternalInput")
    ident = nc.dram_tensor("ident", [_P, _P], bf16, kind="ExternalInput")
    res = nc.dram_tensor("res", [_P, _NCHUNK, 9], f32, kind="ExternalOutput")

    with tile.TileContext(nc) as tc:
        with (
            tc.tile_pool(name="xin", bufs=1) as xin,
            tc.tile_pool(name="work", bufs=1) as work,
            tc.tile_pool(name="junk", bufs=4) as junkp,
            tc.tile_pool(name="psp", bufs=2, space="PSUM") as psp,
        ):
            X = xin.tile([_P, _C, _F], f32)
            T = xin.tile([_P, _F], f8)
            Ident = xin.tile([_P, _P], bf16)
            E = work.tile([_P, _C, _F], bf16)
            LSE = work.tile([_P, _F], bf16)
            lse_acc = work.tile([_P, _NCHUNK], f32)
            acc = work.tile([_P, _NCHUNK, 9], f32)  # [:,h,0]=lse, [:,h,1+c]=pick

            off = 0
            for h, fc in enumerate(_CHUNKS):
                cols = slice(off, off + fc)
                off += fc
                for c in range(_C):
                    nc.sync.dma_start(out=X[:, c, cols], in_=x[c, :, cols])
                    if c == 3:  # T mid-chunk: X classes land sooner, STTs
                        nc.sync.dma_start(out=T[:, cols], in_=t[:, cols])
                    if h == 0 and c == 0:
                        # after X0's issue so the first transfer is X0
                        nc.sync.dma_start(out=Ident, in_=ident[:, :])

                # E_c = exp(X_c); linear chain sum ending at the last class
                # so the post-last-DMA tail is one add + Ln.  Early chunks
                # group classes per Exp op to amortize ACT per-op overhead.
                g = _EXP_GROUPS[h]
                for base in range(0, _C, g):
                    nc.scalar.activation(
                        out=E[:, base : base + g, cols],
                        in_=X[:, base : base + g, cols],
                        func=AF.Exp,
                    )
                # sumexp on the (otherwise idle) PE: PSUM accumulates
                # identity @ E_c over classes, 512-col slices per matmul.
                ps = psp.tile([_P, fc], f32, tag="ps")
                base_col = off - fc
                for c in range(_C):
                    for j in range(0, fc, 512):
                        w = min(512, fc - j)
                        nc.tensor.matmul(
                            out=ps[:, j : j + w],
                            lhsT=Ident,
                            rhs=E[:, c, base_col + j : base_col + j + w],
                            start=(c == 0),
                            stop=(c == _C - 1),
                        )

                # acc[p,h,0] = sum_f log(sumexp)
                nc.scalar.activation(
                    out=LSE[:, cols],
                    in_=ps[:, :],
                    func=AF.Ln,
                    accum_out=lse_acc[:, h : h + 1],
                )

                nc.vector.tensor_copy(
                    out=acc[:, h, 0:1], in_=lse_acc[:, h : h + 1]
                )

                # acc[p,h,1+c] = sum_f (T==c) * X_c  (DVE only)
                for c in range(_C):
                    junk = junkp.tile([_P, fc], bf16, tag="junk")
                    stt = nc.vector.scalar_tensor_tensor(
                        out=junk,
                        in0=T[:, cols],
                        scalar=float(c),
                        in1=X[:, c, cols],
                        op0=OP.is_equal,
                        op1=OP.mult,
                        accum_out=acc[:, h, 1 + c : 2 + c],
                    )
                    del stt  # scheduler interleaves STTs freely

            nc.sync.dma_start(out=res[:, :, :], in_=acc[:, :, :])
    nc.compile()
    return nc


def _exact_host(output, target):
    """Bit-faithful numpy fallback of the reference (only used if the
    top-k == all-nonzeros identity does not hold for these inputs)."""
    x = output.astype(np.float64)
    B, C, H, W = x.shape
    k = int(0.15 * B * H * W)
    m = x.max(axis=1, keepdims=True)
    lse = np.log(np.exp(x - m).sum(axis=1, keepdims=True)) + m
    tt = target.astype(np.int64)
    xt = np.take_along_axis(x, tt[:, None, :, :], axis=1)
    ce = (lse - xt)[:, 0].reshape(B, -1)
    tflat = tt.reshape(B, -1)
    total = 0.0
    n = B * H * W
    for c in range(C):
        row = np.where(tflat == c, ce, 0.0).reshape(-1)
        total += np.partition(row, n - k)[n - k :].sum()
    return np.float32(total / (C * k))


def kernel(output, target):
    import ml_dtypes

    output = np.ascontiguousarray(np.asarray(output), dtype=np.float32)
    target_i = np.asarray(target).astype(np.int64)
    B, C, H, W = output.shape
    k = int(0.15 * B * H * W)
    counts = np.bincount(target_i.ravel(), minlength=C)
    if (B, C, H, W) != (_B, _C, _H, _W) or counts.max() > k:
        return _exact_host(output, target_i)

    global _nc_cache, _last_results
    if _nc_cache is None:
        _nc_cache = _build_nc()

    from concourse.bass_utils import run_bass_kernel_spmd

    xs = output.reshape(B, C, _P, _F)
    ts = target_i.reshape(B, _P, _F).astype(ml_dtypes.float8_e4m3)
    eye = np.ascontiguousarray(np.eye(_P, dtype=ml_dtypes.bfloat16))
    in_maps = [
        {
            "x": np.ascontiguousarray(xs[b]),
            "t": np.ascontiguousarray(ts[b]),
            "ident": eye,
        }
        for b in range(B)
    ]
    r = run_bass_kernel_spmd(
        _nc_cache,
        in_maps,
        core_ids=list(range(_NCORES)),
        trace=_TRACE,
        **_TRACE_KWARGS,
    )
    _last_results = r
    total = 0.0
    for rr in r.results:
        v = rr["res"].astype(np.float64)  # [P, NCHUNK, 9]
        total += (v[:, :, 0] - v[:, :, 1:].sum(axis=2)).sum()
    return np.float32(total / (C * k))
